# revision 8
# baseline (speedup 1.0000x reference)
"""Trainium2 Bass kernel for nn_DeformSpaceAttentionv5 (deformable 3x3 unfold
+ per-channel max + two 1x1 convs + channel-norm dot product).

Contract: kernel(**inputs) takes the FULL inputs (x [4,256,128,128] f32,
offset [4,18,128,128] f32, w0/w1 [256,256] f32, b0/b1 [256] f32) and returns
the FULL output [4,1,128,128] f32.

Strategy (pure data parallel over 8 NeuronCores): core = (batch, H-half).
Per core, the 9 deformable bilinear samples are fetched with SWDGE dma_gather
(fp16, channel-last padded layout, one 2-column window per (sample, y-corner)),
interpolated on DVE in position-major layout with per-partition scalar weights,
max-reduced across the 9 samples, then pushed through PE matmuls
(q^T w0^T / x^T w1^T with an extra channel-sum column) and a fused
normalized-correlation epilogue (ACT square-accumulate + DVE product-accumulate).
"""

import numpy as np

B, C, H, W = 4, 256, 128, 128
PAD = 8
Hp, Wp = H + 2 * PAD, W + 2 * PAD
ROWS = 64            # rows per core (H split in 2)
N = ROWS * W         # positions per core
BLK = 128            # positions per block (= one row)
NBLK = N // BLK      # 64
GRP = 2              # blocks per gather group
NG = NBLK // GRP     # 32
NIDX = GRP * 18 * BLK  # gather indices per group (2 blk * 9 k * 2 rows * 128)
EPS = 1e-5

_NC_CACHE = {}


def _build_nc(has_bias: bool, n_groups: int = NG):
    import concourse.bacc as bacc
    import concourse.bass as bass
    import concourse.tile as tile
    import concourse.mybir as mybir
    from concourse import library_config

    f16 = mybir.dt.float16
    f32 = mybir.dt.float32
    i16 = mybir.dt.int16
    Alu = mybir.AluOpType
    Act = mybir.ActivationFunctionType

    nc = bacc.Bacc("TRN2", target_bir_lowering=False, debug=False, num_devices=8)

    xt = nc.dram_tensor("xt", [Hp * Wp * C], f16, kind="ExternalInput")
    xk = nc.dram_tensor("xk", [2, 128, N], f16, kind="ExternalInput")
    idx = nc.dram_tensor("idx", [n_groups, 128, NIDX // 16], i16, kind="ExternalInput")
    w4 = nc.dram_tensor("w4", [n_groups, 128, GRP, 36], f32, kind="ExternalInput")
    w0t = nc.dram_tensor("w0t", [2, 128, 257], f16, kind="ExternalInput")
    w1t = nc.dram_tensor("w1t", [2, 128, 257], f16, kind="ExternalInput")
    idmat = nc.dram_tensor("idmat", [128, 128], f16, kind="ExternalInput")
    if has_bias:
        qb = nc.dram_tensor("qb", [128, 257], f32, kind="ExternalInput")
        kb = nc.dram_tensor("kb", [128, 257], f32, kind="ExternalInput")
    nblk_t = n_groups * GRP
    o = nc.dram_tensor("o", [128, nblk_t], f32, kind="ExternalOutput")

    # overlapping-window gather view: element j = xt[j*256 : j*256+512]
    xt_view = bass.AP(tensor=xt[:].tensor, offset=0, ap=[[256, Hp * Wp - 1], [1, 512]])

    with tile.TileContext(nc) as tc:
        import contextlib

        with contextlib.ExitStack() as ctx:
            consts = ctx.enter_context(tc.tile_pool(name="consts", bufs=1))
            gpool = ctx.enter_context(tc.tile_pool(name="gath", bufs=2))
            iopool = ctx.enter_context(tc.tile_pool(name="io", bufs=2))
            work = ctx.enter_context(tc.tile_pool(name="work", bufs=2))
            qkpool = ctx.enter_context(tc.tile_pool(name="qk", bufs=2))
            pspool = ctx.enter_context(tc.tile_pool(name="ps", bufs=2, space="PSUM"))

            # constants
            w0t_sb = consts.tile([128, 2, 257], f16)
            nc.sync.dma_start(out=w0t_sb, in_=w0t[:, :, :].rearrange("t p o -> p t o"))
            w1t_sb = consts.tile([128, 2, 257], f16)
            nc.sync.dma_start(out=w1t_sb, in_=w1t[:, :, :].rearrange("t p o -> p t o"))
            ident = consts.tile([128, 128], f16)
            nc.sync.dma_start(out=ident, in_=idmat[:, :])
            if has_bias:
                qb_sb = consts.tile([128, 257], f32)
                nc.sync.dma_start(out=qb_sb, in_=qb[:, :])
                kb_sb = consts.tile([128, 257], f32)
                nc.sync.dma_start(out=kb_sb, in_=kb[:, :])

            # per-block scalar accumulators [128 pos, NBLK]
            sqs = consts.tile([128, nblk_t], f32, tag="sqs")
            sks = consts.tile([128, nblk_t], f32, tag="sks")
            sqks = consts.tile([128, nblk_t], f32, tag="sqks")
            sQs = consts.tile([128, nblk_t], f32, tag="sQs")
            sKs = consts.tile([128, nblk_t], f32, tag="sKs")

            nc.gpsimd.load_library(library_config.mlp)

            for g in range(n_groups):
                idx_t = iopool.tile([128, NIDX // 16], i16, tag="idx")
                nc.sync.dma_start(out=idx_t, in_=idx[g])
                w4_t = iopool.tile([128, GRP, 36], f32, tag="w4")
                nc.sync.dma_start(out=w4_t, in_=w4[g])
                xk_t = iopool.tile([128, 2, GRP * BLK], f16, tag="xk")
                nc.sync.dma_start(
                    out=xk_t, in_=xk[:, :, g * GRP * BLK:(g + 1) * GRP * BLK]
                    .rearrange("t p n -> p t n")
                )
                gat = gpool.tile([128, GRP, 2, 9, 512], f16, tag="gat")
                nc.gpsimd.dma_gather(
                    gat.rearrange("p a b c e -> p (a b c) e"),
                    xt_view, idx_t, NIDX, NIDX, 512, elem_step=256,
                    single_packet=False,
                )

                for blk in range(GRP):
                    nblk = g * GRP + blk
                    q_t = work.tile([128, 256], f16, tag="q")
                    u_t = work.tile([128, 512], f16, tag="u")
                    s_t = work.tile([128, 256], f16, tag="s")
                    for k in range(9):
                        G0 = gat[:, blk, 0, k, :]
                        G1 = gat[:, blk, 1, k, :]
                        wfy1 = w4_t[:, blk, k:k + 1]
                        wfy = w4_t[:, blk, 9 + k:10 + k]
                        wfx1 = w4_t[:, blk, 18 + k:19 + k]
                        wfx = w4_t[:, blk, 27 + k:28 + k]
                        # u = G0*(1-fy) + G1*fy   (512 wide: x0 and x0+1 columns)
                        nc.vector.tensor_scalar(u_t, G0, wfy1, None, Alu.mult)
                        nc.vector.scalar_tensor_tensor(
                            u_t, G1, wfy, u_t, Alu.mult, Alu.add
                        )
                        # s = u0*(1-fx) + u1*fx
                        tgt = q_t if k == 0 else s_t
                        nc.vector.tensor_scalar(
                            tgt, u_t[:, 0:256], wfx1, None, Alu.mult
                        )
                        nc.vector.scalar_tensor_tensor(
                            tgt, u_t[:, 256:512], wfx, tgt, Alu.mult, Alu.add
                        )
                        if k > 0:
                            nc.vector.tensor_tensor(q_t, q_t, s_t, Alu.max)

                    # transpose q -> qT (c-major) via PE
                    qt_ps = pspool.tile([128, 2, 128], f16, tag="qt")
                    for t in range(2):
                        nc.tensor.transpose(
                            qt_ps[:, t, :], q_t[:, t * 128:(t + 1) * 128], ident
                        )
                    qt_sb = work.tile([128, 2, 128], f16, tag="qt_sb")
                    nc.vector.tensor_copy(qt_sb, qt_ps)

                    # Q = qT^T @ w0t  -> [128 pos, 257] (col 256 = sum_o Q)
                    Q_ps = pspool.tile([128, 257], f32, tag="Q")
                    for t in range(2):
                        nc.tensor.matmul(
                            Q_ps, qt_sb[:, t, :], w0t_sb[:, t, :],
                            start=(t == 0), stop=(t == 1),
                        )
                    K_ps = pspool.tile([128, 257], f32, tag="K")
                    for t in range(2):
                        nc.tensor.matmul(
                            K_ps, xk_t[:, t, blk * BLK:(blk + 1) * BLK],
                            w1t_sb[:, t, :], start=(t == 0), stop=(t == 1),
                        )
                    if has_bias:
                        nc.vector.tensor_tensor(Q_ps, Q_ps, qb_sb, Alu.add)
                        nc.vector.tensor_tensor(K_ps, K_ps, kb_sb, Alu.add)

                    # epilogue reductions
                    col = slice(nblk, nblk + 1)
                    act_scr = work.tile([128, 256], f16, tag="act_scr")
                    nc.scalar.activation(
                        act_scr, Q_ps[:, 0:256], Act.Square,
                        accum_out=sqs[:, col],
                    )
                    K_sb = work.tile([128, 256], f16, tag="K_sb")
                    nc.scalar.copy(K_sb, K_ps[:, 0:256])
                    nc.scalar.activation(
                        act_scr, K_sb, Act.Square, accum_out=sks[:, col],
                    )
                    dve_scr = work.tile([128, 256], f16, tag="dve_scr")
                    nc.vector.scalar_tensor_tensor(
                        dve_scr, Q_ps[:, 0:256], 0.0, K_sb, Alu.bypass, Alu.mult,
                        accum_out=sqks[:, col],
                    )
                    nc.vector.tensor_copy(sQs[:, col], Q_ps[:, 256:257])
                    nc.vector.tensor_copy(sKs[:, col], K_ps[:, 256:257])

            # final combine over [128, NBLK]
            tmp = consts.tile([128, nblk_t], f32, tag="tmp")
            num = consts.tile([128, nblk_t], f32, tag="num")
            dq = consts.tile([128, nblk_t], f32, tag="dq")
            dk = consts.tile([128, nblk_t], f32, tag="dk")
            out_t = consts.tile([128, nblk_t], f32, tag="out")
            inv_c = -1.0 / C
            # num = sqk - sQ*sK/C
            nc.vector.tensor_tensor(tmp, sQs, sKs, Alu.mult)
            nc.vector.scalar_tensor_tensor(num, tmp, inv_c, sqks, Alu.mult, Alu.add)
            # dq = sq - sQ^2/C + eps
            nc.vector.tensor_tensor(tmp, sQs, sQs, Alu.mult)
            nc.vector.scalar_tensor_tensor(dq, tmp, inv_c, sqs, Alu.mult, Alu.add)
            nc.vector.tensor_scalar(dq, dq, EPS, None, Alu.add)
            nc.vector.tensor_tensor(tmp, sKs, sKs, Alu.mult)
            nc.vector.scalar_tensor_tensor(dk, tmp, inv_c, sks, Alu.mult, Alu.add)
            nc.vector.tensor_scalar(dk, dk, EPS, None, Alu.add)
            # out = num / sqrt(dq*dk)
            nc.vector.tensor_tensor(tmp, dq, dk, Alu.mult)
            nc.scalar.activation(tmp, tmp, Act.Sqrt)
            nc.vector.reciprocal(tmp, tmp)
            nc.vector.tensor_tensor(out_t, num, tmp, Alu.mult)
            nc.sync.dma_start(out=o[:, :], in_=out_t)

    nc.compile()
    return nc


def _get_nc(has_bias: bool):
    if has_bias not in _NC_CACHE:
        _NC_CACHE[has_bias] = _build_nc(has_bias)
    return _NC_CACHE[has_bias]


def _prep_core(x_b, off_b, h0):
    """Host-side shard prep for one core: indices, weights, fp16 layouts."""
    ys, xs = np.meshgrid(
        np.arange(h0, h0 + ROWS), np.arange(W), indexing="ij"
    )
    ys = ys.reshape(-1).astype(np.float32)
    xs = xs.reshape(-1).astype(np.float32)

    idx_all = np.empty((N, 9), np.int32)
    fy_all = np.empty((N, 9), np.float32)
    fx_all = np.empty((N, 9), np.float32)
    for k in range(9):
        kh, kw = k // 3 - 1, k % 3 - 1
        iy = ys.astype(np.int32)
        ix = xs.astype(np.int32)
        py = ys + kh + off_b[2 * k, iy, ix]
        px = xs + kw + off_b[2 * k + 1, iy, ix]
        y0 = np.clip(np.floor(py).astype(np.int32), -PAD, H + PAD - 2)
        x0 = np.clip(np.floor(px).astype(np.int32), -PAD, W + PAD - 2)
        fy_all[:, k] = py - y0
        fx_all[:, k] = px - x0
        idx_all[:, k] = (y0 + PAD) * Wp + (x0 + PAD)

    # idx tensor [NG, 128, NIDX//16]: slot m = j*128 + p, j = blk*18 + row*9 + k
    idx_np = np.empty((NG, 128, NIDX // 16), np.int16)
    for g in range(NG):
        slots = np.empty((GRP * 18, BLK), np.int32)
        for blk in range(GRP):
            base = (g * GRP + blk) * BLK
            for row in range(2):
                for k in range(9):
                    slots[blk * 18 + row * 9 + k, :] = (
                        idx_all[base:base + BLK, k] + row * Wp
                    )
        wrapped = slots.reshape(-1).reshape(NIDX // 16, 16).T  # [16, cols]
        idx_np[g] = np.tile(wrapped, (8, 1)).astype(np.int16)

    # weights [NG, 128, GRP, 36]
    w4_np = np.empty((NG, 128, GRP, 36), np.float32)
    fy = fy_all.reshape(NBLK, BLK, 9)
    fx = fx_all.reshape(NBLK, BLK, 9)
    for g in range(NG):
        for blk in range(GRP):
            nb = g * GRP + blk
            w4_np[g, :, blk, 0:9] = 1.0 - fy[nb]
            w4_np[g, :, blk, 9:18] = fy[nb]
            w4_np[g, :, blk, 18:27] = 1.0 - fx[nb]
            w4_np[g, :, blk, 27:36] = fx[nb]

    xk_np = np.ascontiguousarray(
        x_b.reshape(2, 128, H, W)[:, :, h0:h0 + ROWS, :].reshape(2, 128, N)
    ).astype(np.float16)
    return idx_np, w4_np, xk_np


def kernel(x, offset, w0, b0, w1, b1):
    from concourse.bass_utils import run_bass_kernel_spmd

    x = np.asarray(x, np.float32)
    offset = np.asarray(offset, np.float32)
    w0 = np.asarray(w0, np.float32)
    w1 = np.asarray(w1, np.float32)
    b0 = np.asarray(b0, np.float32)
    b1 = np.asarray(b1, np.float32)

    has_bias = bool(np.any(b0)) or bool(np.any(b1))
    nc = _get_nc(has_bias)

    w0t_np = np.concatenate([w0.T, w0.sum(0)[:, None]], 1).astype(np.float16)
    w1t_np = np.concatenate([w1.T, w1.sum(0)[:, None]], 1).astype(np.float16)
    w0t_np = np.ascontiguousarray(w0t_np.reshape(2, 128, 257))
    w1t_np = np.ascontiguousarray(w1t_np.reshape(2, 128, 257))

    in_maps = []
    xt_cache = {}
    for core in range(8):
        b, half = core // 2, core % 2
        h0 = ROWS * half
        if b not in xt_cache:
            xp = np.zeros((Hp, Wp, C), np.float16)
            xp[PAD:PAD + H, PAD:PAD + W, :] = x[b].transpose(1, 2, 0)
            xt_cache[b] = xp.reshape(-1)
        idx_np, w4_np, xk_np = _prep_core(x[b], offset[b], h0)
        m = {
            "idmat": np.eye(128, dtype=np.float16),
            "xt": xt_cache[b],
            "xk": xk_np,
            "idx": idx_np,
            "w4": w4_np,
            "w0t": w0t_np,
            "w1t": w1t_np,
        }
        if has_bias:
            qb_np = np.concatenate([b0, [b0.sum()]]).astype(np.float32)
            kb_np = np.concatenate([b1, [b1.sum()]]).astype(np.float32)
            m["qb"] = np.tile(qb_np[None, :], (128, 1))
            m["kb"] = np.tile(kb_np[None, :], (128, 1))
        in_maps.append(m)

    res = run_bass_kernel_spmd(nc, in_maps, core_ids=list(range(8)))

    out = np.empty((B, 1, H, W), np.float32)
    for core in range(8):
        b, half = core // 2, core % 2
        h0 = ROWS * half
        o = res.results[core]["o"]  # [128 pos(x), 64 rows]
        out[b, 0, h0:h0 + ROWS, :] = o.T
    return out
